# revision 1
# baseline (speedup 1.0000x reference)
"""KMeans assignment kernel for TRN2 (8 NeuronCores, data-parallel over points).

Computes argmin_k ||x_n - c_k||^2 for x (65536, 512) f32, centers (4096, 512) f32.

Strategy (single-pass fp32r, ~3x less PE work than the 3-pass hi/lo scheme):
  - argmin_k dist = argmax_k s,  s = 2*x.c_k - ||c_k||^2   (x-norm constant per row)
  - ONE matmul pass p = (2x) @ c^T in fp32r (e8m11, full PE rate). The e8m11
    rounding perturbs each score by sigma ~ 9e-3 while the top1-top2 gap is
    > 0.13 for 99% of points: 18/65536 argmax flips on the actual data
    (rel err 1.1e-2, under the 2e-2 gate).
  - The -||c_k||^2 bias enters as the START matmul of each bank's PSUM
    accumulation group: ones[2,128] stationary x [bias_hi; bias_lo] moving
    (hi/lo fp32r split keeps the bias exact to ~6e-5). A proper start=True
    group is the only PSUM-init the tile scheduler orders correctly —
    engine-side preloads race the PE (verified: nondeterministic cold-start
    corruption; manual semaphore repairs can deadlock the device).
  - Per half (4 banks): bias matmuls back-to-back (ones stationary loads
    once), then fc-outer so each x chunk stays stationary across 4 banks.
  - Act evacuates each PSUM half to a s[128,2,4,512] SBUF tile (releases
    PSUM early; no PE stall on WAR).
  - Argmax: DVE tensor_reduce -> 8 block maxes, top8 sorts them (global max
    at slot 0), then ONE max_index scan over s finds the first occurrence =
    exact argmin index with jnp-compatible tie-breaking.
  - Data-parallel: 8192 points/core, centers replicated; no collectives.
"""
import os
import numpy as np

import concourse.bass as bass
import concourse.bacc as bacc
import concourse.tile as tile
import concourse.mybir as mybir
from concourse.bass_utils import run_bass_kernel_spmd

N_CORES = 8
N_POINTS = 65536
K = 4096
F = 512
PTS_PER_CORE = N_POINTS // N_CORES      # 8192
NT = PTS_PER_CORE // 128                # 64 x-tiles per core
NFC = F // 128                          # 4 contraction chunks
NQ = 4                                  # PSUM quarters
KQ = K // NQ                            # 1024 centers per quarter
F32 = mybir.dt.float32
F32R = mybir.dt.float32r
U32 = mybir.dt.uint32
ALU = mybir.AluOpType

_NC = None
LAST_BR = None


def round_fp32r(a: np.ndarray) -> np.ndarray:
    """Round f32 to fp32r (e8m11): RNE to 11 mantissa bits; low 12 bits zero."""
    bits = np.ascontiguousarray(a, dtype=np.float32).view(np.uint32)
    rounded = (bits.astype(np.uint64) + 0x7FF + ((bits >> 12) & 1)) & 0xFFFFF000
    return rounded.astype(np.uint32).view(np.float32)


def _build():
    nc = bacc.Bacc("TRN2", target_bir_lowering=False, debug=False,
                   num_devices=N_CORES)
    xh_d = nc.declare_dram_parameter("xh", [NT, 128, NFC, 128], F32R, isOutput=False)
    ch_d = nc.declare_dram_parameter("ch", [NFC, 128, K], F32R, isOutput=False)
    cnn_d = nc.declare_dram_parameter("cnn", [2, K], F32R, isOutput=False)
    one_d = nc.declare_dram_parameter("one2", [2, 128], F32R, isOutput=False)
    out_d = nc.declare_dram_parameter("oidx", [128, NT], U32, isOutput=True)

    NB = 4                              # banks per PSUM half
    with tile.TileContext(nc) as tc:
        with (
            tc.tile_pool(name="const", bufs=1) as cpool,
            tc.tile_pool(name="xp", bufs=4) as xpool,
            tc.tile_pool(name="sp", bufs=3) as spool,
            tc.tile_pool(name="m8p", bufs=2) as m8pool,
            tc.tile_pool(name="st", bufs=1) as stpool,
            tc.tile_pool(name="ps", bufs=1, space="PSUM") as pspool,
        ):
            # [bias_hi; bias_lo] fp32r rows of -||c||^2; the hi/lo pair keeps
            # the bias exact to ~6e-5 despite the 11-bit fp32r mantissa.
            cnn = cpool.tile([2, K], F32R, tag="cnn")
            nc.sync.dma_start(cnn[:], cnn_d[:])
            ones2 = cpool.tile([2, 128], F32R, tag="ones2")
            nc.sync.dma_start(ones2[:], one_d[:])
            # Spread the 4x4MB center loads across four DMA trigger queues so
            # they land in ~1/4 the serial time (the first tile's matmuls wait
            # on all of them).
            chs = []
            ch_engines = [nc.scalar, nc.gpsimd, nc.scalar, nc.gpsimd]
            for fc in range(NFC):
                cht = cpool.tile([128, K], F32R, tag=f"ch{fc}", name=f"ch{fc}")
                ch_engines[fc].dma_start(cht[:], ch_d[fc])
                chs.append(cht)

            ist = stpool.tile([128, NT, 8], U32, tag="ist")

            for t in range(NT):
                xt = xpool.tile([128, NFC * 128], F32R, tag="x")
                nc.sync.dma_start(xt[:], xh_d[t])

                s = spool.tile([128, 2, NB, 512], F32, tag="s")
                for h in range(2):
                    ph = pspool.tile([128, NB, 512], F32, tag=f"p{h}",
                                     name=f"p{h}")
                    def ks(b):
                        return slice(h * 2048 + b * 512, h * 2048 + (b + 1) * 512)
                    # Bias matmuls open each bank's accumulation group: a
                    # proper start=True group is the only PSUM-init the tile
                    # scheduler tracks (engine preloads race the PE). Grouped
                    # back-to-back so the ones-stationary is loaded once.
                    for b in range(NB):
                        nc.tensor.matmul(
                            ph[:, b, :], ones2[:], cnn[:, ks(b)],
                            start=True, stop=False)
                    # fc-outer: each xt chunk stays stationary across 4 banks.
                    for fc in range(NFC):
                        for b in range(NB):
                            nc.tensor.matmul(
                                ph[:, b, :],
                                xt[:, fc * 128:(fc + 1) * 128],
                                chs[fc][:, ks(b)],
                                start=False,
                                stop=(fc == NFC - 1),
                            )
                    # evacuate PSUM half to SBUF (frees it for tile t+1)
                    nc.scalar.copy(s[:, h], ph[:])

                # DVE: block maxes -> sorted top8 -> ONE exact argmax scan.
                # The reduce is split per half so it overlaps the other
                # half's evacuation (shortens the final-tile drain chain).
                m8 = m8pool.tile([128, 8], F32, tag="m8")
                m8s = m8pool.tile([128, 8], F32, tag="m8s")
                for h in range(2):
                    nc.vector.tensor_reduce(
                        out=m8[:, h * 4:(h + 1) * 4], in_=s[:, h],
                        axis=mybir.AxisListType.X, op=ALU.max)
                nc.vector.max(m8s[:], m8[:])
                nc.vector.max_index(ist[:, t, :], m8s[:],
                                    s.rearrange("p h b f -> p (h b f)"))

            ex = stpool.tile([128, NT], U32, tag="ex")
            nc.vector.tensor_copy(out=ex[:], in_=ist[:, :, 0])
            nc.gpsimd.dma_start(out_d[:], ex[:])
    nc.compile()
    return nc


def _get_nc():
    global _NC
    if _NC is None:
        _NC = _build()
    return _NC


def kernel(x: np.ndarray, centers: np.ndarray) -> np.ndarray:
    global LAST_BR, _LAST_IN_MAPS
    x = np.ascontiguousarray(x, dtype=np.float32)
    centers = np.ascontiguousarray(centers, dtype=np.float32)

    v_hi = round_fp32r((2.0 * x).astype(np.float32))
    c_hi = round_fp32r(centers)

    # pack x side: [core, t, fp, fc, j] <- v[core*8192 + t*128 + j, fc*128 + fp]
    a = v_hi.reshape(N_CORES, NT, 128, NFC, 128)      # [core, t, j, fc, fp]
    xh_p = np.ascontiguousarray(a.transpose(0, 1, 4, 3, 2))

    # pack c side: [fc, fp, k] <- c[k, fc*128 + fp]
    c = c_hi.reshape(K, NFC, 128)                     # [k, fc, fp]
    ch_p = np.ascontiguousarray(c.transpose(1, 2, 0))

    bias = (-(centers.astype(np.float64) ** 2).sum(axis=1)).astype(np.float32)
    b_hi = round_fp32r(bias)
    b_lo = round_fp32r((bias - b_hi).astype(np.float32))
    cnn_p = np.ascontiguousarray(np.stack([b_hi, b_lo], axis=0))  # (2, K)

    one2 = np.ones((2, 128), dtype=np.float32)
    in_maps = [
        {"xh": xh_p[i], "ch": ch_p, "cnn": cnn_p, "one2": one2}
        for i in range(N_CORES)
    ]

    nc = _get_nc()
    _LAST_IN_MAPS = in_maps
    br = run_bass_kernel_spmd(nc, in_maps, list(range(N_CORES)))
    LAST_BR = br

    parts = []
    for i in range(N_CORES):
        oidx = br.results[i]["oidx"]                  # (128, NT) u32
        parts.append(oidx.T.reshape(-1))              # point-major
    return np.concatenate(parts).astype(np.int32)


_LAST_IN_MAPS = None


def _install_ntff_shim():
    """antenv.axon_hooks is missing in some images; rebuild it from the boot
    helper so run_bass_kernel_spmd(trace=True) can profile via NTFF."""
    import sys, types
    try:
        from antenv.axon_hooks import get_axon_ntff_profile_hook  # noqa: F401
        return True
    except ImportError:
        pass
    try:
        from trn_agent_boot.trn_boot import _ntff_profile_via_ctypes
        hook = _ntff_profile_via_ctypes('/opt/axon/libaxon_pjrt.so')
        mod = types.ModuleType("antenv.axon_hooks")
        mod.get_axon_ntff_profile_hook = lambda: hook
        mod.set_axon_ntff_profile_hook = lambda h: None
        sys.modules["antenv.axon_hooks"] = mod
        return True
    except Exception:
        return False


def measure_exec_ns(reps: int = 3) -> int:
    """Real HW execution time from a neuron-profile (NTFF) capture; falls
    back to best-of-N wall clock around the execute if profiling is
    unavailable."""
    import tempfile
    import time
    nc = _get_nc()
    assert _LAST_IN_MAPS is not None, "call kernel() first"
    try:
        _install_ntff_shim()
        tmpdir = tempfile.mkdtemp(prefix="kmeans_ntff_")
        br = run_bass_kernel_spmd(nc, _LAST_IN_MAPS, list(range(N_CORES)),
                                  trace=True, tmpdir=tmpdir)
        if br.exec_time_ns is not None:
            return int(br.exec_time_ns)
    except Exception:
        pass
    best = None
    for _ in range(max(1, reps)):
        t0 = time.perf_counter()
        run_bass_kernel_spmd(nc, _LAST_IN_MAPS, list(range(N_CORES)))
        dt = time.perf_counter() - t0
        best = dt if best is None else min(best, dt)
    return int(best * 1e9)



# revision 12
# speedup vs baseline: 1.1280x; 1.1280x over previous
"""KMeans assignment kernel for TRN2 (8 NeuronCores, data-parallel over points).

Computes argmin_k ||x_n - c_k||^2 for x (65536, 512) f32, centers (4096, 512) f32.

Strategy v4 (fp16 matmul + per-half DVE max + Act fp16 gap + DVE 4x select):
  - argmin_k dist = argmax_k s,  s = 2*x.c_k - ||c_k||^2   (x-norm constant per row)
  - ONE matmul pass p = (2x) @ c^T in fp16. 16-bit moving data streams 2
    cols/cycle through the PE vs fp32r's 1 (measured 255ns/MM fp32r at N=512,
    ~131ns fp16). fp16 operand rounding: 37/65536 argmax flips measured on the
    actual data (rel err 1.60e-2, under the 2e-2 gate).
  - Bias -||c||^2 enters as hi/lo fp16 rows via the bank's start=True matmul
    pair (ones[2,128] stationary), the baseline-proven PSUM-init pattern; the
    fp16 lo row keeps the bias exact to ~1.2e-4. (A fused tensor_tensor_reduce
    evacuation was tried instead and NRT_EXEC_UNIT_UNRECOVERABLE-faults this
    HW build in every variant - avoid.)
  - Per half (4 banks): DVE tensor_reduce max from PSUM -> m_h; Act
    g16_h = fp16(Identity(-s + m_h)) >= 0 with == +0 exactly at the half's
    argmax (per-partition bias port carries m_h, scale=-1; fp16 subnormals
    keep gaps down to 6e-8 away from 0). PSUM bank group frees after the Act
    read - no cross-half dependency pins it.
  - DVE index extraction per half in 4x_2p mode (all operands fp16/SBUF, 0.25
    cyc/elem): junk = (g16 <= 0) * iota16, accum = sum = argmax position in
    the half (iota16 = 0..2047, fp16-exact).
  - Host picks the winning half per point from the two f32 maxes (m1 > m0 ->
    half 1, ties -> half 0 = jnp's first-index tiebreak) and combines
    idx = 2048*h + a_h. Ties within fp16 gap resolution (<6e-8) are the only
    corruption source: measured 0 on this data.
  - Engine budget/tile: PE 40 MMs ~5.2us, DVE 4.8 (reduces) + 1.6 (selects),
    Act ~4.1us -> DVE-paced ~6.4us/tile vs 10.7us for the fp32r baseline.
  - Data-parallel: 8192 points/core, centers replicated; no collectives.
"""
import os
import numpy as np

import concourse.bass as bass
import concourse.bacc as bacc
import concourse.tile as tile
import concourse.mybir as mybir
from concourse.bass_utils import run_bass_kernel_spmd

N_CORES = 8
N_POINTS = 65536
K = 4096
F = 512
PTS_PER_CORE = N_POINTS // N_CORES      # 8192
NT = PTS_PER_CORE // 128                # 64 x-tiles per core
NFC = F // 128                          # 4 contraction chunks
NB = 4                                  # banks per PSUM half
KH = K // 2                             # 2048 centers per half
F32 = mybir.dt.float32
F16 = mybir.dt.float16
ALU = mybir.AluOpType
AF = mybir.ActivationFunctionType

_NC = None
LAST_BR = None


def _build():
    nc = bacc.Bacc("TRN2", target_bir_lowering=False, debug=False,
                   num_devices=N_CORES)
    xh_d = nc.declare_dram_parameter("xh", [NT, 128, NFC, 128], F16, isOutput=False)
    ch_d = nc.declare_dram_parameter("ch", [NFC, 2, 128, KH], F16, isOutput=False)
    cnn_d = nc.declare_dram_parameter("cnn", [2, K], F16, isOutput=False)
    one_d = nc.declare_dram_parameter("one2", [2, 128], F16, isOutput=False)
    iota_d = nc.declare_dram_parameter("iotar", [128, KH], F16, isOutput=False)
    oa_d = nc.declare_dram_parameter("oacc", [128, NT, 2], F32, isOutput=True)
    om_d = nc.declare_dram_parameter("omax", [128, NT, 2], F32, isOutput=True)

    with tile.TileContext(nc) as tc:
        with (
            tc.tile_pool(name="const", bufs=1) as cpool,
            tc.tile_pool(name="xp", bufs=4) as xpool,
            tc.tile_pool(name="gp", bufs=3) as gpool,
            tc.tile_pool(name="jk", bufs=1) as jkpool,
            tc.tile_pool(name="st", bufs=1) as stpool,
            tc.tile_pool(name="ps", bufs=1, space="PSUM") as pspool,
        ):
            # Prologue DMAs in first-consumption order on the gpsimd trigger
            # queue (Act runs the g16 passes; sync streams x).
            cnn = cpool.tile([2, K], F16, tag="cnn")
            nc.gpsimd.dma_start(cnn[:], cnn_d[:])
            ones2 = cpool.tile([2, 128], F16, tag="ones2")
            nc.gpsimd.dma_start(ones2[:], one_d[:])
            chs = [[None, None] for _ in range(NFC)]
            for h in range(2):
                for fc in range(NFC):
                    cht = cpool.tile([128, KH], F16, tag=f"ch{fc}_{h}",
                                     name=f"ch{fc}_{h}")
                    nc.gpsimd.dma_start(cht[:], ch_d[fc, h])
                    chs[fc][h] = cht
            iotat = cpool.tile([128, KH], F16, tag="iotar")
            nc.gpsimd.dma_start(iotat[:], iota_d[:])

            ast = stpool.tile([128, NT, 2], F32, tag="ast")
            mst = stpool.tile([128, NT, 2], F32, tag="mst")

            for t in range(NT):
                xt = xpool.tile([128, NFC * 128], F16, tag="x")
                nc.sync.dma_start(xt[:], xh_d[t])

                for h in range(2):
                    ph = pspool.tile([128, NB, 512], F32, tag=f"p{h}",
                                     name=f"p{h}")
                    def ks(b):
                        return slice(b * 512, (b + 1) * 512)
                    # Bias rows open each bank's accumulation group
                    # (start=True is the only PSUM init the scheduler orders
                    # correctly); grouped so the ones-stationary loads once.
                    for b in range(NB):
                        nc.tensor.matmul(
                            ph[:, b, :], ones2[:],
                            cnn[:, h * KH + b * 512:h * KH + (b + 1) * 512],
                            start=True, stop=False)
                    for fc in range(NFC):
                        for b in range(NB):
                            nc.tensor.matmul(
                                ph[:, b, :],
                                xt[:, fc * 128:(fc + 1) * 128],
                                chs[fc][h][:, ks(b)],
                                start=False,
                                stop=(fc == NFC - 1),
                            )
                    # Exact f32 max of the half, straight from PSUM.
                    nc.vector.tensor_reduce(
                        out=mst[:, t, h:h + 1],
                        in_=ph.rearrange("p b f -> p (b f)"),
                        axis=mybir.AxisListType.X,
                        op=ALU.max,
                    )
                    # Act: g16_h = fp16(m_h - s) >= 0, == +0 only at the
                    # half's argmax; reading PSUM frees the bank group.
                    g16 = gpool.tile([128, KH], F16, tag="g16")
                    nc.scalar.activation(
                        out=g16[:],
                        in_=ph.rearrange("p b f -> p (b f)"),
                        func=AF.Identity,
                        bias=mst[:, t, h:h + 1],
                        scale=-1.0,
                    )
                    # DVE 4x_2p select: accum = sum((g16 <= 0) * iota16)
                    #                         = argmax position in the half.
                    junk = jkpool.tile([128, KH], F16, tag="junk")
                    nc.vector.scalar_tensor_tensor(
                        out=junk[:],
                        in0=g16[:],
                        scalar=0.0,
                        in1=iotat[:],
                        op0=ALU.is_le,
                        op1=ALU.mult,
                        accum_out=ast[:, t, h:h + 1],
                    )

            nc.sync.dma_start(oa_d[:], ast[:])
            nc.sync.dma_start(om_d[:], mst[:])
    nc.compile()
    return nc


def _get_nc():
    global _NC
    if _NC is None:
        _NC = _build()
    return _NC


def kernel(x: np.ndarray, centers: np.ndarray) -> np.ndarray:
    global LAST_BR, _LAST_IN_MAPS
    x = np.ascontiguousarray(x, dtype=np.float32)
    centers = np.ascontiguousarray(centers, dtype=np.float32)

    v16 = (2.0 * x).astype(np.float16)
    c16 = centers.astype(np.float16)

    # pack x side: [core, t, fp, fc, j] <- v[core*8192 + t*128 + j, fc*128 + fp]
    a = v16.reshape(N_CORES, NT, 128, NFC, 128)       # [core, t, j, fc, fp]
    xh_p = np.ascontiguousarray(a.transpose(0, 1, 4, 3, 2))

    # pack c side: [fc, h, fp, kh] <- c[h*2048 + kh, fc*128 + fp]
    c = c16.reshape(2, KH, NFC, 128)                  # [h, kh, fc, fp]
    ch_p = np.ascontiguousarray(c.transpose(2, 0, 3, 1))

    # bias -||c||^2 as hi/lo fp16 rows (lo keeps it exact to ~1.2e-4)
    bias = -(centers.astype(np.float64) ** 2).sum(axis=1)
    b_hi = bias.astype(np.float16)
    b_lo = (bias - b_hi.astype(np.float64)).astype(np.float16)
    cnn_p = np.stack([b_hi, b_lo], axis=0)            # (2, K) f16

    one2 = np.ones((2, 128), dtype=np.float16)
    iota_p = np.ascontiguousarray(np.broadcast_to(
        np.arange(KH).astype(np.float16)[None, :], (128, KH)))

    in_maps = [
        {"xh": xh_p[i], "ch": ch_p, "cnn": cnn_p, "one2": one2,
         "iotar": iota_p}
        for i in range(N_CORES)
    ]

    nc = _get_nc()
    _LAST_IN_MAPS = in_maps
    br = run_bass_kernel_spmd(nc, in_maps, list(range(N_CORES)))
    LAST_BR = br

    parts = []
    for i in range(N_CORES):
        acc = br.results[i]["oacc"]                   # (128, NT, 2) f32
        mm = br.results[i]["omax"]                    # (128, NT, 2) f32
        hstar = (mm[:, :, 1] > mm[:, :, 0]).astype(np.int64)
        a_h = np.where(hstar == 1, acc[:, :, 1], acc[:, :, 0]).astype(np.int64)
        idx = hstar * KH + a_h                        # (128, NT)
        parts.append(idx.T.reshape(-1))               # point-major
    return np.concatenate(parts).astype(np.int32)


_LAST_IN_MAPS = None


def _install_ntff_shim():
    """antenv.axon_hooks is missing in some images; rebuild it from the boot
    helper so run_bass_kernel_spmd(trace=True) can profile via NTFF."""
    import sys, types
    try:
        from antenv.axon_hooks import get_axon_ntff_profile_hook  # noqa: F401
        return True
    except ImportError:
        pass
    try:
        from trn_agent_boot.trn_boot import _ntff_profile_via_ctypes
        hook = _ntff_profile_via_ctypes('/opt/axon/libaxon_pjrt.so')
        mod = types.ModuleType("antenv.axon_hooks")
        mod.get_axon_ntff_profile_hook = lambda: hook
        mod.set_axon_ntff_profile_hook = lambda h: None
        sys.modules["antenv.axon_hooks"] = mod
        return True
    except Exception:
        return False


def measure_exec_ns(reps: int = 3) -> int:
    """Real HW execution time from a neuron-profile (NTFF) capture; falls
    back to best-of-N wall clock around the execute if profiling is
    unavailable."""
    import tempfile
    import time
    nc = _get_nc()
    assert _LAST_IN_MAPS is not None, "call kernel() first"
    try:
        _install_ntff_shim()
        tmpdir = tempfile.mkdtemp(prefix="kmeans_ntff_")
        br = run_bass_kernel_spmd(nc, _LAST_IN_MAPS, list(range(N_CORES)),
                                  trace=True, tmpdir=tmpdir)
        if br.exec_time_ns is not None:
            return int(br.exec_time_ns)
    except Exception:
        pass
    best = None
    for _ in range(max(1, reps)):
        t0 = time.perf_counter()
        run_bass_kernel_spmd(nc, _LAST_IN_MAPS, list(range(N_CORES)))
        dt = time.perf_counter() - t0
        best = dt if best is None else min(best, dt)
    return int(best * 1e9)


# revision 20
# speedup vs baseline: 1.1519x; 1.0212x over previous
"""KMeans assignment kernel for TRN2 (8 NeuronCores, data-parallel over points).

Computes argmin_k ||x_n - c_k||^2 for x (65536, 512) f32, centers (4096, 512) f32.

Strategy v4 (fp16 matmul + per-half DVE max + Act fp16 gap + DVE 4x select):
  - argmin_k dist = argmax_k s,  s = 2*x.c_k - ||c_k||^2   (x-norm constant per row)
  - ONE matmul pass p = (2x) @ c^T in fp16. 16-bit moving data streams 2
    cols/cycle through the PE vs fp32r's 1 (measured 255ns/MM fp32r at N=512,
    ~131ns fp16). fp16 operand rounding: 37/65536 argmax flips measured on the
    actual data (rel err 1.60e-2, under the 2e-2 gate).
  - Bias -||c||^2 enters as hi/lo fp16 rows via the bank's start=True matmul
    pair (ones[2,128] stationary), the baseline-proven PSUM-init pattern; the
    fp16 lo row keeps the bias exact to ~1.2e-4. (A fused tensor_tensor_reduce
    evacuation was tried instead and NRT_EXEC_UNIT_UNRECOVERABLE-faults this
    HW build in every variant - avoid.)
  - Per half (4 banks): DVE tensor_reduce max from PSUM -> m_h; Act
    g16_h = fp16(Identity(-s + m_h)) >= 0 with == +0 exactly at the half's
    argmax (per-partition bias port carries m_h, scale=-1; fp16 subnormals
    keep gaps down to 6e-8 away from 0). PSUM bank group frees after the Act
    read - no cross-half dependency pins it.
  - DVE index extraction per half in 4x_2p mode (all operands fp16/SBUF, 0.25
    cyc/elem): junk = (g16 <= 0) * iota16, accum = sum = argmax position in
    the half (iota16 = 0..2047, fp16-exact).
  - Host picks the winning half per point from the two f32 maxes (m1 > m0 ->
    half 1, ties -> half 0 = jnp's first-index tiebreak) and combines
    idx = 2048*h + a_h. Ties within fp16 gap resolution (<6e-8) are the only
    corruption source: measured 0 on this data.
  - Engine budget/tile: PE 40 MMs ~5.2us, DVE 4.8 (reduces) + 1.6 (selects),
    Act ~4.1us -> DVE-paced ~6.4us/tile vs 10.7us for the fp32r baseline.
  - Data-parallel: 8192 points/core, centers replicated; no collectives.
"""
import os
import numpy as np

import concourse.bass as bass
import concourse.bacc as bacc
import concourse.tile as tile
import concourse.mybir as mybir
from concourse.bass_utils import run_bass_kernel_spmd

N_CORES = 8
N_POINTS = 65536
K = 4096
F = 512
PTS_PER_CORE = N_POINTS // N_CORES      # 8192
NT = PTS_PER_CORE // 128                # 64 x-tiles per core
NFC = F // 128                          # 4 contraction chunks
NB = 4                                  # banks per PSUM half
KH = K // 2                             # 2048 centers per half
F32 = mybir.dt.float32
F16 = mybir.dt.float16
ALU = mybir.AluOpType
AF = mybir.ActivationFunctionType

_NC = None
LAST_BR = None


def _build():
    nc = bacc.Bacc("TRN2", target_bir_lowering=False, debug=False,
                   num_devices=N_CORES)
    xh_d = nc.declare_dram_parameter("xh", [NT, 128, NFC, 128], F16, isOutput=False)
    ch_d = nc.declare_dram_parameter("ch", [NFC, 2, 128, KH], F16, isOutput=False)
    cnn_d = nc.declare_dram_parameter("cnn", [128, K], F16, isOutput=False)
    one_d = nc.declare_dram_parameter("one2", [128, 128], F16, isOutput=False)
    iota_d = nc.declare_dram_parameter("iotar", [128, KH], F16, isOutput=False)
    oa_d = nc.declare_dram_parameter("oacc", [128, NT, 2], F16, isOutput=True)
    om_d = nc.declare_dram_parameter("omax", [128, NT, 2], F32, isOutput=True)

    with tile.TileContext(nc) as tc:
        with (
            tc.tile_pool(name="const", bufs=1) as cpool,
            tc.tile_pool(name="xp", bufs=4) as xpool,
            tc.tile_pool(name="gp", bufs=3) as gpool,
            tc.tile_pool(name="jk", bufs=1) as jkpool,
            tc.tile_pool(name="st", bufs=1) as stpool,
            tc.tile_pool(name="ps", bufs=1, space="PSUM") as pspool,
        ):
            # Prologue DMAs in first-consumption order on the gpsimd trigger
            # queue (Act runs the g16 passes; sync streams x).
            # Bias rows replicated at partition offsets 32b so the 4 bias
            # matmuls of a half row-tile-pack into the PE array concurrently
            # (tile_position row groups 0/32/64/96; each has contraction 2).
            cnn = cpool.tile([128, K], F16, tag="cnn")
            nc.gpsimd.dma_start(cnn[:], cnn_d[:])
            ones2 = cpool.tile([128, 128], F16, tag="ones2")
            nc.gpsimd.dma_start(ones2[:], one_d[:])
            chs = [[None, None] for _ in range(NFC)]
            for h in range(2):
                for fc in range(NFC):
                    cht = cpool.tile([128, KH], F16, tag=f"ch{fc}_{h}",
                                     name=f"ch{fc}_{h}")
                    nc.gpsimd.dma_start(cht[:], ch_d[fc, h])
                    chs[fc][h] = cht
            iotat = cpool.tile([128, KH], F16, tag="iotar")
            nc.gpsimd.dma_start(iotat[:], iota_d[:])

            ast = stpool.tile([128, NT, 2], F16, tag="ast")
            mst = stpool.tile([128, NT, 2], F32, tag="mst")

            for t in range(NT):
                xt = xpool.tile([128, NFC * 128], F16, tag="x")
                nc.sync.dma_start(xt[:], xh_d[t])

                for h in range(2):
                    ph = pspool.tile([128, NB, 512], F32, tag=f"p{h}",
                                     name=f"p{h}")
                    def ks(b):
                        return slice(b * 512, (b + 1) * 512)
                    # Bias rows open each bank's accumulation group
                    # (start=True is the only PSUM init the scheduler orders
                    # correctly). Banks 0-2 sit in distinct 32-row
                    # tile_position groups so they run concurrently in the
                    # array (AP base_partition allows only 0/32/64); bank 3
                    # reuses group 0 and serializes behind bank 0.
                    for b in range(NB):
                        r = 32 * (b % 3)
                        nc.tensor.matmul(
                            ph[:, b, :], ones2[r:r + 2, :],
                            cnn[r:r + 2,
                                h * KH + b * 512:h * KH + (b + 1) * 512],
                            start=True, stop=False)
                    for fc in range(NFC):
                        for b in range(NB):
                            nc.tensor.matmul(
                                ph[:, b, :],
                                xt[:, fc * 128:(fc + 1) * 128],
                                chs[fc][h][:, ks(b)],
                                start=False,
                                stop=(fc == NFC - 1),
                            )
                    # Exact f32 max of the half, straight from PSUM.
                    nc.vector.tensor_reduce(
                        out=mst[:, t, h:h + 1],
                        in_=ph.rearrange("p b f -> p (b f)"),
                        axis=mybir.AxisListType.X,
                        op=ALU.max,
                    )
                    # Act: g16_h = fp16(m_h - s) >= 0, == +0 only at the
                    # half's argmax; reading PSUM frees the bank group.
                    g16 = gpool.tile([128, KH], F16, tag="g16")
                    nc.scalar.activation(
                        out=g16[:],
                        in_=ph.rearrange("p b f -> p (b f)"),
                        func=AF.Identity,
                        bias=mst[:, t, h:h + 1],
                        scale=-1.0,
                    )
                    # DVE 4x_2p select: accum = sum((g16 <= 0) * iota16)
                    #                         = argmax position in the half.
                    junk = jkpool.tile([128, KH], F16, tag="junk")
                    nc.vector.scalar_tensor_tensor(
                        out=junk[:],
                        in0=g16[:],
                        scalar=0.0,
                        in1=iotat[:],
                        op0=ALU.is_le,
                        op1=ALU.mult,
                        accum_out=ast[:, t, h:h + 1],
                    )

            nc.sync.dma_start(oa_d[:], ast[:])
            nc.sync.dma_start(om_d[:], mst[:])
    nc.compile()
    return nc


def _get_nc():
    global _NC
    if _NC is None:
        _NC = _build()
    return _NC


def kernel(x: np.ndarray, centers: np.ndarray) -> np.ndarray:
    global LAST_BR, _LAST_IN_MAPS
    x = np.ascontiguousarray(x, dtype=np.float32)
    centers = np.ascontiguousarray(centers, dtype=np.float32)

    v16 = (2.0 * x).astype(np.float16)
    c16 = centers.astype(np.float16)

    # pack x side: [core, t, fp, fc, j] <- v[core*8192 + t*128 + j, fc*128 + fp]
    a = v16.reshape(N_CORES, NT, 128, NFC, 128)       # [core, t, j, fc, fp]
    xh_p = np.ascontiguousarray(a.transpose(0, 1, 4, 3, 2))

    # pack c side: [fc, h, fp, kh] <- c[h*2048 + kh, fc*128 + fp]
    c = c16.reshape(2, KH, NFC, 128)                  # [h, kh, fc, fp]
    ch_p = np.ascontiguousarray(c.transpose(2, 0, 3, 1))

    # bias -||c||^2 as hi/lo fp16 rows (lo keeps it exact to ~1.2e-4),
    # replicated at partition offsets 32b for the row-tiled bias matmuls
    bias = -(centers.astype(np.float64) ** 2).sum(axis=1)
    b_hi = bias.astype(np.float16)
    b_lo = (bias - b_hi.astype(np.float64)).astype(np.float16)
    cnn_p = np.zeros((128, K), dtype=np.float16)
    one2 = np.zeros((128, 128), dtype=np.float16)
    for b in range(3):
        cnn_p[32 * b] = b_hi
        cnn_p[32 * b + 1] = b_lo
        one2[32 * b] = 1.0
        one2[32 * b + 1] = 1.0
    iota_p = np.ascontiguousarray(np.broadcast_to(
        np.arange(KH).astype(np.float16)[None, :], (128, KH)))

    in_maps = [
        {"xh": xh_p[i], "ch": ch_p, "cnn": cnn_p, "one2": one2,
         "iotar": iota_p}
        for i in range(N_CORES)
    ]

    nc = _get_nc()
    _LAST_IN_MAPS = in_maps
    br = run_bass_kernel_spmd(nc, in_maps, list(range(N_CORES)))
    LAST_BR = br

    parts = []
    for i in range(N_CORES):
        acc = br.results[i]["oacc"].astype(np.float32)  # (128, NT, 2)
        mm = br.results[i]["omax"]                    # (128, NT, 2) f32
        hstar = (mm[:, :, 1] > mm[:, :, 0]).astype(np.int64)
        a_h = np.where(hstar == 1, acc[:, :, 1], acc[:, :, 0]).astype(np.int64)
        idx = hstar * KH + a_h                        # (128, NT)
        parts.append(idx.T.reshape(-1))               # point-major
    return np.concatenate(parts).astype(np.int32)


_LAST_IN_MAPS = None


def _install_ntff_shim():
    """antenv.axon_hooks is missing in some images; rebuild it from the boot
    helper so run_bass_kernel_spmd(trace=True) can profile via NTFF."""
    import sys, types
    try:
        from antenv.axon_hooks import get_axon_ntff_profile_hook  # noqa: F401
        return True
    except ImportError:
        pass
    try:
        from trn_agent_boot.trn_boot import _ntff_profile_via_ctypes
        hook = _ntff_profile_via_ctypes('/opt/axon/libaxon_pjrt.so')
        mod = types.ModuleType("antenv.axon_hooks")
        mod.get_axon_ntff_profile_hook = lambda: hook
        mod.set_axon_ntff_profile_hook = lambda h: None
        sys.modules["antenv.axon_hooks"] = mod
        return True
    except Exception:
        return False


def measure_exec_ns(reps: int = 3) -> int:
    """Real HW execution time from a neuron-profile (NTFF) capture; falls
    back to best-of-N wall clock around the execute if profiling is
    unavailable."""
    import tempfile
    import time
    nc = _get_nc()
    assert _LAST_IN_MAPS is not None, "call kernel() first"
    try:
        _install_ntff_shim()
        tmpdir = tempfile.mkdtemp(prefix="kmeans_ntff_")
        br = run_bass_kernel_spmd(nc, _LAST_IN_MAPS, list(range(N_CORES)),
                                  trace=True, tmpdir=tmpdir)
        if br.exec_time_ns is not None:
            return int(br.exec_time_ns)
    except Exception:
        pass
    best = None
    for _ in range(max(1, reps)):
        t0 = time.perf_counter()
        run_bass_kernel_spmd(nc, _LAST_IN_MAPS, list(range(N_CORES)))
        dt = time.perf_counter() - t0
        best = dt if best is None else min(best, dt)
    return int(best * 1e9)


# revision 21
# speedup vs baseline: 1.4588x; 1.2664x over previous
"""KMeans assignment kernel for TRN2 (8 NeuronCores, data-parallel over points).

Computes argmin_k ||x_n - c_k||^2 for x (65536, 512) f32, centers (4096, 512) f32.

Strategy v6 (fp16 matmul + ONE custom fused bias+argmax DVE op per half):
  - argmin_k dist = argmax_k s,  s = 2*x.c_k - ||c_k||^2   (x-norm constant per row)
  - ONE matmul pass p = (2x) @ c^T in fp16 (measured ~222ns/512-col MM vs
    255ns fp32r - the fp16 gain is the 2x faster FWL weight load; the array
    streams 1 col/cycle for every dtype). fp16 operand rounding: 37/65536
    argmax flips measured on the actual data (rel err 1.60e-2, under the
    2e-2 gate). NO bias matmuls (each costs a full 512 cols = 20% PE).
  - Per half (4 banks), ONE custom DVE instruction (registered at import into
    concourse's per-NEFF custom-op table; no firmware change) reads PSUM and
    computes, in a single 1-elem/cycle pass:
        s    = Src0 + Src1              # p + bias (exact f32 bias, Src1 full tensor)
        r    = scan(MAX, s)             # running max
        out  = select(s == r, Idx, r)   # Idx at prefix-maxima (>=0), else r (<0)
        accum= MAX(out)                 # last prefix-max position = argmax_k s
    Scores are always negative (s <= -50 at 8.5 sigma) so Idx >= 0 dominates
    r in the accum, and out[:, -1] = r[-1] = the half's max value m_h -
    unless the argmax IS the last element (accum == 2047), in which case the
    host recomputes those ~2/2048 points exactly (tiny numpy matmul).
    (The stock tensor_tensor_reduce NRT-faults this HW build in every
    variant; max8/max_index/tensor_reduce+select need 2 full DVE passes.)
  - Act does only the two [128,1] copies of out[:, -1] into the m staging
    tile. Host picks the winning half per point (m1 > m0, ties -> half 0 =
    jnp's first-index tiebreak): idx = 2048*h + j_h.
  - Engine budget/tile: PE 32 MMs ~7.2us (bottleneck ~96% busy), DVE 2
    fused passes ~5.1us, Act ~0.6us -> ~465us vs 688us fp32r baseline.
  - Data-parallel: 8192 points/core, centers replicated; no collectives.
"""
import os
import numpy as np

import concourse.bass as bass
import concourse.bacc as bacc
import concourse.tile as tile
import concourse.mybir as mybir
from concourse.bass_utils import run_bass_kernel_spmd

N_CORES = 8
N_POINTS = 65536
K = 4096
F = 512
PTS_PER_CORE = N_POINTS // N_CORES      # 8192
NT = PTS_PER_CORE // 128                # 64 x-tiles per core
NFC = F // 128                          # 4 contraction chunks
NB = 4                                  # banks per PSUM half
KH = K // 2                             # 2048 centers per half
F32 = mybir.dt.float32
F16 = mybir.dt.float16

_NC = None
LAST_BR = None
_ARGMAX_OP = None


def _get_argmax_op():
    """Register (once) the fused bias-add + running-max + argmax custom DVE
    op in concourse's custom-op registry. The uop program is per-NEFF table
    data; shas are computed here so the pin always matches this build."""
    global _ARGMAX_OP
    if _ARGMAX_OP is not None:
        return _ARGMAX_OP
    import concourse.dve_ops as dve_ops_mod
    from concourse.dve_ops import DveOp, OPS
    from concourse.dve_spec import (
        Spec, Src0, Src1, AluOp, Idx, scan, eq, select, lower,
    )
    from concourse.dve_uop import DveOpSpec

    name = "ARGMAX_BIAS_ANT"
    for op in OPS:
        if op.name == name:
            _ARGMAX_OP = op
            return op

    s = Src0 + Src1
    r = scan(AluOp.MAX, s)
    body = select(eq(s, r), Idx, r)

    def ref(in0, in1, s0, s1, imm2):
        P = in0.shape[0]
        ss = (np.asarray(in0, np.float32).reshape(P, -1)
              + np.asarray(in1, np.float32).reshape(P, -1))
        rr = np.maximum.accumulate(ss, axis=1)
        idx = np.broadcast_to(
            np.arange(ss.shape[1], dtype=np.float32), ss.shape)
        out = np.where(ss == rr, idx, rr).astype(np.float32)
        return out.reshape(in0.shape), out.max(axis=1)

    spec = Spec(body=body, accum=AluOp.MAX, reference=ref)
    row = dve_ops_mod._CUSTOM_DVE_ROW_BASE + len(OPS)
    shas = {}
    for ver in ("v3", "v4"):
        op_spec = DveOpSpec(name=name, opcode=row,
                            uops=lower(spec, ver=ver), rd1_en=True)
        shas[ver] = op_spec.sha(ver)
    op = DveOp(name, spec, subdim=False, uops_sha=shas)
    OPS.append(op)
    dve_ops_mod.CUSTOM_DVE_SPECS[name] = spec
    dve_ops_mod._SUB_OPCODE_FOR_NAME[name] = row
    _ARGMAX_OP = op
    return op


def _build():
    argmax_op = _get_argmax_op()
    nc = bacc.Bacc("TRN2", target_bir_lowering=False, debug=False,
                   num_devices=N_CORES)
    xh_d = nc.declare_dram_parameter("xh", [NT, 128, NFC, 128], F16, isOutput=False)
    ch_d = nc.declare_dram_parameter("ch", [NFC, 2, 128, KH], F16, isOutput=False)
    bias_d = nc.declare_dram_parameter("biasr", [2, 128, KH], F32, isOutput=False)
    oa_d = nc.declare_dram_parameter("oacc", [128, NT, 2], F32, isOutput=True)
    om_d = nc.declare_dram_parameter("omax", [128, NT, 2], F32, isOutput=True)

    with tile.TileContext(nc) as tc:
        with (
            tc.tile_pool(name="const", bufs=1) as cpool,
            tc.tile_pool(name="xp", bufs=4) as xpool,
            tc.tile_pool(name="jk", bufs=3) as jkpool,
            tc.tile_pool(name="st", bufs=1) as stpool,
            tc.tile_pool(name="ps", bufs=1, space="PSUM") as pspool,
        ):
            # Prologue DMAs in first-consumption order on the gpsimd queue.
            chs = [[None, None] for _ in range(NFC)]
            biast = cpool.tile([128, 2, KH], F32, tag="biasr")
            for h in range(2):
                for fc in range(NFC):
                    cht = cpool.tile([128, KH], F16, tag=f"ch{fc}_{h}",
                                     name=f"ch{fc}_{h}")
                    nc.gpsimd.dma_start(cht[:], ch_d[fc, h])
                    chs[fc][h] = cht
                nc.gpsimd.dma_start(biast[:, h], bias_d[h])

            ast = stpool.tile([128, NT, 2], F32, tag="ast")
            mst = stpool.tile([128, NT, 2], F32, tag="mst")

            for t in range(NT):
                xt = xpool.tile([128, NFC * 128], F16, tag="x")
                nc.sync.dma_start(xt[:], xh_d[t])

                for h in range(2):
                    ph = pspool.tile([128, NB, 512], F32, tag=f"p{h}",
                                     name=f"p{h}")
                    for fc in range(NFC):
                        for b in range(NB):
                            nc.tensor.matmul(
                                ph[:, b, :],
                                xt[:, fc * 128:(fc + 1) * 128],
                                chs[fc][h][:, b * 512:(b + 1) * 512],
                                start=(fc == 0),
                                stop=(fc == NFC - 1),
                            )
                    # ONE fused pass: bias-add + running max + argmax accum.
                    junk = jkpool.tile([128, KH], F32, tag="junk")
                    nc.vector._custom_dve(
                        argmax_op,
                        out=junk[:],
                        in0=ph.rearrange("p b f -> p (b f)"),
                        in1=biast[:, h],
                        accum_out=ast[:, t, h:h + 1],
                    )
                    # m_h = the running max's final value (valid unless the
                    # argmax is the last element - host repairs those).
                    nc.scalar.copy(mst[:, t, h:h + 1], junk[:, KH - 1:KH])

            nc.sync.dma_start(oa_d[:], ast[:])
            nc.sync.dma_start(om_d[:], mst[:])
    nc.compile()
    return nc


def _get_nc():
    global _NC
    if _NC is None:
        _NC = _build()
    return _NC


def kernel(x: np.ndarray, centers: np.ndarray) -> np.ndarray:
    global LAST_BR, _LAST_IN_MAPS
    x = np.ascontiguousarray(x, dtype=np.float32)
    centers = np.ascontiguousarray(centers, dtype=np.float32)

    v16 = (2.0 * x).astype(np.float16)
    c16 = centers.astype(np.float16)

    # pack x side: [core, t, fp, fc, j] <- v[core*8192 + t*128 + j, fc*128 + fp]
    a = v16.reshape(N_CORES, NT, 128, NFC, 128)       # [core, t, j, fc, fp]
    xh_p = np.ascontiguousarray(a.transpose(0, 1, 4, 3, 2))

    # pack c side: [fc, h, fp, kh] <- c[h*2048 + kh, fc*128 + fp]
    c = c16.reshape(2, KH, NFC, 128)                  # [h, kh, fc, fp]
    ch_p = np.ascontiguousarray(c.transpose(2, 0, 3, 1))

    bias = (-(centers.astype(np.float64) ** 2).sum(axis=1)).astype(np.float32)
    bias_p = np.ascontiguousarray(
        np.broadcast_to(bias.reshape(2, 1, KH), (2, 128, KH)))

    in_maps = [
        {"xh": xh_p[i], "ch": ch_p, "biasr": bias_p}
        for i in range(N_CORES)
    ]

    nc = _get_nc()
    _LAST_IN_MAPS = in_maps
    br = run_bass_kernel_spmd(nc, in_maps, list(range(N_CORES)))
    LAST_BR = br

    idx_all = np.empty((N_CORES, 128, NT), dtype=np.int64)
    repair = []                                       # (core, p, t) triples
    for i in range(N_CORES):
        acc = br.results[i]["oacc"].astype(np.int64)  # (128, NT, 2) j_h
        mm = br.results[i]["omax"]                    # (128, NT, 2) m_h
        hstar = (mm[:, :, 1] > mm[:, :, 0]).astype(np.int64)
        j_h = np.where(hstar == 1, acc[:, :, 1], acc[:, :, 0])
        idx_all[i] = hstar * KH + j_h
        bad = np.nonzero((acc[:, :, 0] == KH - 1) | (acc[:, :, 1] == KH - 1))
        repair.extend((i, int(p), int(t)) for p, t in zip(*bad))

    if repair:
        # argmax at a half's last slot -> that half's max value is unknown
        # (the running-max output was overwritten by the hit index).
        # Recompute those few points exactly (same fp16-quantized math).
        pts = np.array([core * PTS_PER_CORE + t * 128 + p
                        for core, p, t in repair], dtype=np.int64)
        sc = (v16[pts].astype(np.float32) @ c16.T.astype(np.float32)
              + bias[None, :])
        fixed = np.argmax(sc, axis=1)
        for (core, p, t), f in zip(repair, fixed):
            idx_all[core, p, t] = int(f)

    parts = [idx_all[i].T.reshape(-1) for i in range(N_CORES)]
    return np.concatenate(parts).astype(np.int32)


_LAST_IN_MAPS = None


def _install_ntff_shim():
    """antenv.axon_hooks is missing in some images; rebuild it from the boot
    helper so run_bass_kernel_spmd(trace=True) can profile via NTFF."""
    import sys, types
    try:
        from antenv.axon_hooks import get_axon_ntff_profile_hook  # noqa: F401
        return True
    except ImportError:
        pass
    try:
        from trn_agent_boot.trn_boot import _ntff_profile_via_ctypes
        hook = _ntff_profile_via_ctypes('/opt/axon/libaxon_pjrt.so')
        mod = types.ModuleType("antenv.axon_hooks")
        mod.get_axon_ntff_profile_hook = lambda: hook
        mod.set_axon_ntff_profile_hook = lambda h: None
        sys.modules["antenv.axon_hooks"] = mod
        return True
    except Exception:
        return False


def measure_exec_ns(reps: int = 3) -> int:
    """Real HW execution time from a neuron-profile (NTFF) capture; falls
    back to best-of-N wall clock around the execute if profiling is
    unavailable."""
    import tempfile
    import time
    nc = _get_nc()
    assert _LAST_IN_MAPS is not None, "call kernel() first"
    try:
        _install_ntff_shim()
        tmpdir = tempfile.mkdtemp(prefix="kmeans_ntff_")
        br = run_bass_kernel_spmd(nc, _LAST_IN_MAPS, list(range(N_CORES)),
                                  trace=True, tmpdir=tmpdir)
        if br.exec_time_ns is not None:
            return int(br.exec_time_ns)
    except Exception:
        pass
    best = None
    for _ in range(max(1, reps)):
        t0 = time.perf_counter()
        run_bass_kernel_spmd(nc, _LAST_IN_MAPS, list(range(N_CORES)))
        dt = time.perf_counter() - t0
        best = dt if best is None else min(best, dt)
    return int(best * 1e9)
